# revision 19
# baseline (speedup 1.0000x reference)
"""Trainium2 Bass kernel for nn_CNN3_P (dense_cnn), 8-core data parallel.

Network (per sample):
  x [128,64] -> pairwise conv -> relu -> [256,127]
  -> conv1d k3 (x3, relu) -> [256,121] -> FC 30976->512 relu -> FC 512->1

Strategy: batch 2048 split 256/core. Channels on partitions (2 chunks of
128); all layers run on a flat [128, T*stride] layout where the K=3 conv
shifts are plain column offsets; boundary columns hold garbage that
never reaches valid outputs. All matmuls in fp16 (1 cyc/row on the PE),
PSUM accumulates fp32. Conv3 output is stored (s, l)-major so FC1's
stationary operands are contiguous.

Changes vs the 652.7us flat-128 version (now ~640us):

Progressive stride compaction: each
evac writes only the valid positions at a narrower per-sample stride
(x/pairwise 127 -> conv1 125 -> conv2 123 -> conv3 121), so conv
matmuls never re-process boundary-garbage columns. Saves ~27K PE
column-cycles (~11us) over the uniform stride-128 layout. Conv
evacuations alternate DVE/ACT by output chunk so both nt=1 chunks
evacuate concurrently at pipeline fill. The FC2 endgame ships the
sc=0 y-half early as a single-descriptor row DMA (a partition-major
[128,1] DMA is 128 descriptors whose completion bookkeeping adds
~3us to the runtime finalize window), and sc=0 defers fewer trailing
FC1 l-values (HOLD0=6 vs HOLD1=10) so its DVE dot-product chain
hides entirely under sc=1's deferred matmuls.

Schedule notes: x is packed host-side into one j-major dram tensor so
each t-tile is a single fully contiguous DMA (the first tile's DMA is
split across two queues to shorten the critical path to the first
matmul); all weight dram layouts are partition-major so weight DMAs
are contiguous; the Wf1 stream pool (bufs=4) is opened alongside the
conv pools in disjoint SBUF, so its first groups prefetch during the
conv phase instead of stalling the PE ~6us at the conv->FC transition
and the stream stays ahead of FC1; pairwise(1) is hoisted before the
conv loop to cover the conv1-weight DMA; the sc=0 FC1 psum group
closes ~2us before sc=1 (HOLD trailing l-values) so FC2-sc0 overlaps
FC1-sc1 matmuls, and the h4 activation is sliced per-128 so the FC2
transposes pipeline off the PE sooner. The FC2 identity is DMA'd on
the gpsimd queue instead of built with gpsimd DIRECT2D ops, which
otherwise run at the head of the gpsimd queue and delay the first
x-tile DMA.

fp8 was evaluated and rejected: DoubleRow fp8 matmuls cost the same
wall time as fp16 for the same output columns (2x MACs, measured
218ns for 256x128x512), and e4m3's ~5% rms/operand error gives ~12%
output error vs the 2e-2 gate, so neither plain nor hi/lo compensated
fp8 can win. Winograd F(2,3) was evaluated and rejected: the output
transform's tensor_tensor adds can only run on the DVE (ACT is
single-input), and at 1 elem/cyc/lane x 0.96 GHz on PSUM operands the
added DVE work exceeds the PE cycles saved.
"""
import os
import sys

for _p in ('/opt/trn_rl_repo', '/root/.axon_site/_ro/trn_rl_repo'):
    if os.path.isdir(_p) and _p not in sys.path:
        sys.path.insert(0, _p)

import numpy as np

import concourse.bacc as bacc
import concourse.mybir as mybir
import concourse.tile as tile
from concourse.bass_utils import run_bass_kernel_spmd

F32 = mybir.dt.float32
F16 = mybir.dt.float16

P = 128
CL = 128          # context length
IL = 64           # inst length
PC = 256          # channels (all layers)
NCHUNK = 2        # channel chunks of 128
LF = 121          # conv3 valid positions
F1 = 512
N_CORES = 8
B = 2048
BCORE = B // N_CORES      # 256
T = 8                     # samples per conv sub-tile
NT = BCORE // T           # 32
# per-sample strides after each stage's compacting evacuation
S0 = 127                  # x pack / pairwise output
S1 = 125                  # conv1 output
S2 = 123                  # conv2 output
S3 = 121                  # conv3 output (== LF)
H0 = T * S0               # 1016
H1 = T * S1               # 1000
H2 = T * S2               # 984
W0 = 4 * S0               # 508  pairwise/conv1 psum width
W1 = 4 * S1               # 500  conv2 psum width
W2 = 4 * S2               # 492  conv3 psum width
NTC = 2                   # psum tiles per t-tile
SPT = 4                   # samples per psum tile
SC = BCORE // P           # 2 sample chunks of 128 for FC
GL = 11                   # l-slices per Wf1 DMA group (121 = 11*11)


def build_nc():
    nc = bacc.Bacc("TRN2", target_bir_lowering=False, debug=False)

    x_d = nc.dram_tensor("xfull", [P, BCORE, S0], F16, kind="ExternalInput")
    wpc_d = nc.dram_tensor("wpc", [P, PC], F16, kind="ExternalInput")
    bp_d = nc.dram_tensor("bpc", [NCHUNK, P], F32, kind="ExternalInput")
    wc_d = [nc.dram_tensor(f"w{i}t", [NCHUNK, P, 3, NCHUNK, P], F16,
                           kind="ExternalInput") for i in (1, 2, 3)]
    bc_d = [nc.dram_tensor(f"b{i}c", [NCHUNK, P], F32, kind="ExternalInput")
            for i in (1, 2, 3)]
    wf1_d = nc.dram_tensor("wf1t", [NCHUNK, P, LF, F1], F16, kind="ExternalInput")
    bf1_d = nc.dram_tensor("bf1r", [1, F1], F16, kind="ExternalInput")
    ones_d = nc.dram_tensor("onesr", [1, P], F16, kind="ExternalInput")
    wf2_d = nc.dram_tensor("wf2p", [4, P, P], F16, kind="ExternalInput")
    wf2b_d = nc.dram_tensor("wf2b", [P, F1], F16, kind="ExternalInput")
    bf2_d = nc.dram_tensor("bf2s", [1, 1], F32, kind="ExternalInput")
    ident_d = nc.dram_tensor("identr", [P, P], F16, kind="ExternalInput")
    y_d = nc.dram_tensor("y", [BCORE, 1], F32, kind="ExternalOutput")

    RELU = mybir.ActivationFunctionType.Relu

    with tile.TileContext(nc) as tc:
        with tc.tile_pool(name="const", bufs=1) as cpool, \
             tc.tile_pool(name="h3c", bufs=1) as h3pool, \
             tc.tile_pool(name="wf1", bufs=4) as wfpool:
            # --- constants / weights, resident all kernel ---
            wpc = cpool.tile([P, PC], F16)
            nc.sync.dma_start(wpc[:], wpc_d.ap())
            bp = cpool.tile([P, NCHUNK], F32)
            nc.sync.dma_start(bp[:], bp_d.ap().rearrange("c p -> p c"))
            bf1 = cpool.tile([1, F1], F16)
            nc.sync.dma_start(bf1[:], bf1_d.ap())
            ones = cpool.tile([1, P], F16)
            nc.sync.dma_start(ones[:], ones_d.ap())
            # conv weights: per layer, per ci-chunk: [ci, (k, coc, co)]
            wconv = []
            for i in range(3):
                tiles = []
                for cic in range(NCHUNK):
                    w = cpool.tile([P, 3 * NCHUNK * P], F16, tag=f"w{i}_{cic}")
                    nc.sync.dma_start(
                        w[:].rearrange("p (k b c) -> p k b c", k=3, b=NCHUNK),
                        wc_d[i].ap()[cic])
                    tiles.append(w)
                wconv.append(tiles)
            bconv = []
            for i in range(3):
                bt = cpool.tile([P, NCHUNK], F32, tag=f"bc{i}")
                nc.sync.dma_start(bt[:], bc_d[i].ap().rearrange("c p -> p c"))
                bconv.append(bt)
            bf2 = cpool.tile([1, 1], F32)
            nc.sync.dma_start(bf2[:], bf2_d.ap())

            # persistent conv3 output, fp16, (s, l)-major: col = s*S3 + l
            h3c = [h3pool.tile([P, BCORE * S3], F16, tag=f"h3c{cc}", name=f"h3c{cc}")
                   for cc in range(NCHUNK)]
            h3v = [h.rearrange("p (s l) -> p s l", l=S3) for h in h3c]

            # ---------------- conv phase ----------------
            with tc.tile_pool(name="xt", bufs=3) as xtpool, \
                 tc.tile_pool(name="h", bufs=2) as hpool, \
                 tc.tile_pool(name="psP", bufs=2, space="PSUM") as psP, \
                 tc.tile_pool(name="ps1", bufs=2, space="PSUM") as ps1p, \
                 tc.tile_pool(name="ps2", bufs=2, space="PSUM") as ps2p, \
                 tc.tile_pool(name="ps3", bufs=2, space="PSUM") as ps3p:
                NTS = list(range(NTC - 1, -1, -1))   # nt=1 first: its consumers
                # don't cross the nt boundary, so they unblock earliest

                # HAM pre-warm: the PE clock idles throttled at 1.2 GHz and
                # only reaches 2.4 GHz after ~3.4us of sustained activity.
                # The first x/weight DMAs cannot land before ~9us (engine
                # boot + trigger latency), so burn that window on dummy
                # matmuls over a memset scratch tile (8 x N=508 at ~423ns
                # cold = 3.4us exactly, ending as the first x operands
                # land; more/smaller dummies either undershoot the window
                # or get interleaved after the first real matmul),
                # so the real matmuls start at full clock instead of paying
                # ~2.5us of half-rate warmup.
                with tc.high_priority():
                    scratch = cpool.tile([P, W0], F16, tag="warm")
                    nc.gpsimd.memset(scratch[:], 0.0)
                    wps = psP.tile([P, W0], F32, tag="ps", name="warmps")
                    for _ in range(8):
                        nc.tensor.matmul(wps[:], scratch[:, 0:P], scratch[:],
                                         start=True, stop=True)

                def pairwise(t):
                    xt = xtpool.tile([P, H0], F16, tag="xt", name="xt")
                    if t == 0:
                        # first tile: split across the gpsimd + scalar
                        # queues (sync is busy with the weight triggers).
                        # The nt=1 half (samples 4-7) feeds the first
                        # matmul, so it rides gpsimd, which lands before
                        # the scalar queue clears its ACT_TABLE_LOAD.
                        xv = xt[:].rearrange("p (s i) -> p s i", i=S0)
                        nc.gpsimd.dma_start(xv[:, T // 2:T],
                                            x_d.ap()[:, T // 2:T, :])
                        nc.scalar.dma_start(xv[:, 0:T // 2],
                                            x_d.ap()[:, 0:T // 2, :])
                    else:
                        nc.gpsimd.dma_start(
                            xt[:].rearrange("p (s i) -> p s i", i=S0),
                            x_d.ap()[:, t * T:(t + 1) * T, :])
                    h0 = [hpool.tile([P, H0], F16, tag=f"h0_{cc}", bufs=4,
                                     name=f"h0_{cc}") for cc in range(NCHUNK)]
                    for nt in NTS:
                        for cc in range(NCHUNK):
                            ps = psP.tile([P, W0], F32, tag="ps", name="pwps")
                            sl_ = slice(nt * W0, (nt + 1) * W0)
                            nc.tensor.matmul(ps[:], wpc[:, cc * P:(cc + 1) * P],
                                             xt[:, sl_], start=True, stop=True)
                            nc.scalar.activation(h0[cc][:, sl_], ps[:],
                                                 RELU, bias=bp[:, cc:cc + 1])
                    return h0

                def conv_layer(hin, w_tiles, pool, hlen, wout, evac):
                    # group-outer: each psum group completes early so its
                    # evacuation overlaps the remaining groups' matmuls
                    for nt in NTS:
                        for co in range(NCHUNK):
                            ps = pool.tile([P, wout], F32,
                                           tag="ps", name=f"cps{co}_{nt}")
                            step = 0
                            for k in range(3):
                                for ci in range(NCHUNK):
                                    lhsT = w_tiles[ci][:, (k * NCHUNK + co) * P:
                                                       (k * NCHUNK + co + 1) * P]
                                    nk = min(wout, hlen - nt * wout - k)
                                    nc.tensor.matmul(
                                        ps[:, 0:nk], lhsT,
                                        hin[ci][:, nt * wout + k:
                                                nt * wout + k + nk],
                                        start=(step == 0), stop=(step == 5))
                                    step += 1
                            evac(co, nt, ps)

                h0_next = pairwise(0)
                # hoist pairwise(1): its matmuls give the PE ~0.9us of work
                # while the conv1 weight DMA finishes at kernel start
                h0_after = pairwise(1)
                for t in range(NT):
                    h0 = h0_next
                    h0_next = h0_after
                    # h1/h2 are produced and consumed within one t-iteration,
                    # so a single buffer suffices (h0 needs 3: pairwise runs
                    # two tiles ahead)
                    h1 = [hpool.tile([P, H1], F16, tag=f"h1_{cc}", bufs=1,
                                     name=f"h1_{cc}") for cc in range(NCHUNK)]

                    def evac1(co, nt, ps):
                        out = (h1[co][:].rearrange("p (s l) -> p s l", l=S1)
                               [:, nt * SPT:(nt + 1) * SPT, :])
                        src = (ps[:].rearrange("p (s l) -> p s l", l=S0)
                               [:, :, 0:S1])
                        if co == 0:
                            nc.vector.tensor_scalar(
                                out, src, bconv[0][:, co:co + 1], 0.0,
                                mybir.AluOpType.add, mybir.AluOpType.max)
                        else:
                            nc.scalar.activation(out, src, RELU,
                                                 bias=bconv[0][:, co:co + 1])
                    conv_layer(h0, wconv[0], ps1p, H0, W0, evac1)

                    # emit the next-but-one tile's pairwise here so its
                    # evacuations age a full tile before conv1 consumes them
                    if t + 2 < NT:
                        h0_after = pairwise(t + 2)

                    h2 = [hpool.tile([P, H2], F16, tag=f"h2_{cc}", bufs=1,
                                     name=f"h2_{cc}") for cc in range(NCHUNK)]

                    def evac2(co, nt, ps):
                        out = (h2[co][:].rearrange("p (s l) -> p s l", l=S2)
                               [:, nt * SPT:(nt + 1) * SPT, :])
                        src = (ps[:].rearrange("p (s l) -> p s l", l=S1)
                               [:, :, 0:S2])
                        if co == 0:
                            nc.vector.tensor_scalar(
                                out, src, bconv[1][:, co:co + 1], 0.0,
                                mybir.AluOpType.add, mybir.AluOpType.max)
                        else:
                            nc.scalar.activation(out, src, RELU,
                                                 bias=bconv[1][:, co:co + 1])
                    conv_layer(h1, wconv[1], ps2p, H1, W1, evac2)

                    def evac3(co, nt, ps, t=t):
                        s0_ = t * T + nt * SPT
                        out = h3v[co][:, s0_:s0_ + SPT, :]
                        src = (ps[:].rearrange("p (s l) -> p s l", l=S2)
                               [:, :, 0:S3])
                        if co == 0:
                            nc.scalar.activation(out, src, RELU,
                                                 bias=bconv[2][:, co:co + 1])
                        else:
                            nc.vector.tensor_scalar(
                                out, src, bconv[2][:, co:co + 1], 0.0,
                                mybir.AluOpType.add, mybir.AluOpType.max)
                    conv_layer(h2, wconv[2], ps3p, H2, W2, evac3)

            # ---------------- FC phase ----------------
            # (conv pools closed; wfpool stays open from before the conv
            # phase so its first group DMAs prefetch during conv)
            with tc.tile_pool(name="h4", bufs=1) as h4pool, \
                 tc.tile_pool(name="f2ps", bufs=2, space="PSUM") as f2pspool:
                # FC2-only constants (not needed until ~100us into the FC
                # phase), loaded into SBUF freed by the conv pools; gpsimd
                # queue (drained by now) so they don't sit behind the
                # blocked wf1 group DMAs on the sync queue
                wf2 = h4pool.tile([P, 4 * P], F16, tag="wf2")
                nc.gpsimd.dma_start(wf2[:].rearrange("p (f m) -> p f m", f=4),
                                    wf2_d.ap().rearrange("f p m -> p f m"))
                wf2b = h4pool.tile([P, F1], F16, tag="wf2b")
                nc.gpsimd.dma_start(wf2b[:], wf2b_d.ap())
                ident = h4pool.tile([P, P], F16, tag="ident")
                nc.gpsimd.dma_start(ident[:], ident_d.ap())
                ps_fc1 = [f2pspool.tile([P, F1], F32, tag=f"fc1ps{sc}", bufs=1,
                                        name=f"fc1ps{sc}") for sc in range(SC)]
                for sc in range(SC):
                    nc.tensor.matmul(ps_fc1[sc][:], ones[:], bf1[:],
                                     start=True, stop=False)
                # HOLD = trailing l-values run sc-serialized so sc=0's psum
                # closes early and its DVE FC2 chain mostly overlaps sc=1's
                # deferred matmuls. HOLD=14 (a window that fully covers the
                # ~2.4us chain) measured ~1us SLOWER than 10 — don't "fix"
                # the remaining transpose wait by widening this. sc=0
                # defers fewer (HOLD0) so its ~1.9us DVE chain starts
                # earlier and the ytp transpose is ready before sc=1's
                # deferred matmuls retire.
                HOLD = {0: 6, 1: 12}
                deferred = {0: [], 1: []}
                for cc in range(NCHUNK):
                    for lg in range(LF // GL):
                        rw = wfpool.tile([P, GL * F1], F16, tag="wf1")
                        nc.sync.dma_start(
                            rw[:].rearrange("p (l f) -> p l f", l=GL),
                            wf1_d.ap()[cc][:, lg * GL:(lg + 1) * GL, :])
                        for ll in range(GL):
                            l = lg * GL + ll
                            for sc in range(SC):
                                if (cc == NCHUNK - 1
                                        and l >= LF - HOLD[sc]):
                                    deferred[sc].append((cc, l, rw, ll))
                                    continue
                                nc.tensor.matmul(
                                    ps_fc1[sc][:],
                                    h3v[cc][:, sc * P:(sc + 1) * P, l],
                                    rw[:, ll * F1:(ll + 1) * F1],
                                    start=False, stop=False)
                ystage = h4pool.tile([1, BCORE], F32, tag="ystage")

                def fc1_tail(sc):
                    for i, (cc, l, rw, ll) in enumerate(deferred[sc]):
                        nc.tensor.matmul(
                            ps_fc1[sc][:],
                            h3v[cc][:, sc * P:(sc + 1) * P, l],
                            rw[:, ll * F1:(ll + 1) * F1],
                            start=False, stop=(i == len(deferred[sc]) - 1))

                # --- sc=0: FC2 dot product on the DVE; everything except
                # the tiny gather transpose overlaps sc=1's deferred
                # matmuls, and its y-half ships early as a single-
                # descriptor DMA (a partition-major [128,1] DMA is 128
                # descriptors whose completion bookkeeping adds ~3us to
                # the runtime finalize window -- don't) ---
                fc1_tail(0)
                h40 = h4pool.tile([P, F1], F16, tag="h4_0", name="h4_0")
                nc.scalar.activation(h40[:], ps_fc1[0][:], RELU)
                prod = h4pool.tile([P, F1], F32, tag="pr0")
                nc.vector.tensor_tensor(prod[:], h40[:], wf2b[:],
                                        mybir.AluOpType.mult)
                ysum = h4pool.tile([P, 1], F32, tag="ys0")
                nc.vector.tensor_reduce(ysum[:], prod[:], mybir.AxisListType.X,
                                        mybir.AluOpType.add)
                y16 = h4pool.tile([P, 1], F16, tag="y16")
                nc.vector.tensor_copy(y16[:], ysum[:])

                fc1_tail(1)
                # sc=0 gather: [128,1] -> [1,128] on the PE (~30ns, operands
                # ready long before sc=1's last matmul retires)
                ytp = f2pspool.tile([1, P], F16, tag="ytp", bufs=1)
                nc.tensor.transpose(ytp[:], y16[:], ident[:])
                nc.vector.tensor_scalar_add(ystage[:, 0:P], ytp[0:1, :], bf2[:])
                nc.sync.dma_start(y_d.ap().rearrange("b one -> one b")
                                  [:, 0:P], ystage[:, 0:P])
                # --- sc=1: FC2 via PE transposes (shortest tail chain); the
                # activation is sliced per-128, alternating ACT/DVE so two
                # slices evacuate concurrently, and each transpose starts
                # as soon as its slice is ready ---
                h41 = h4pool.tile([P, F1], F16, tag="h4_1", name="h4_1")
                h4t = h4pool.tile([P, 4 * P], F16, tag="h4t_1", name="h4t_1")
                for fc in range(4):
                    sl_ = slice(fc * P, (fc + 1) * P)
                    if fc % 2 == 0:
                        nc.scalar.activation(h41[:, sl_], ps_fc1[1][:, sl_],
                                             RELU)
                    else:
                        nc.vector.tensor_scalar(h41[:, sl_], ps_fc1[1][:, sl_],
                                                0.0, 0.0, mybir.AluOpType.add,
                                                mybir.AluOpType.max)
                    tp = f2pspool.tile([P, P], F16, tag="fc2tp", bufs=2)
                    nc.tensor.transpose(tp[:], h41[:, sl_], ident[:])
                    if fc % 2 == 0:
                        nc.vector.tensor_copy(h4t[:, sl_], tp[:])
                    else:
                        nc.scalar.copy(h4t[:, sl_], tp[:])
                po = f2pspool.tile([P, P], F32, tag="fc2ps", bufs=1)
                for fc in range(4):
                    nc.tensor.matmul(po[:], wf2[:, fc * P:(fc + 1) * P],
                                     h4t[:, fc * P:(fc + 1) * P],
                                     start=(fc == 0), stop=(fc == 3))
                nc.vector.tensor_scalar_add(ystage[:, P:2 * P],
                                            po[0:1, :], bf2[:])
                nc.sync.dma_start(y_d.ap().rearrange("b one -> one b")
                                  [:, P:2 * P], ystage[:, P:2 * P])

    nc.compile()
    return nc


_NC_CACHE = None


def _get_nc():
    global _NC_CACHE
    if _NC_CACHE is None:
        _NC_CACHE = build_nc()
    return _NC_CACHE


def prep_inputs(x, Wp, bp, W1, b1, W2, b2, W3, b3, Wf1, bf1, Wf2, bf2):
    """Host-side shard + weight re-layout. Returns per-core input maps."""
    f32, f16 = np.float32, np.float16
    wp = np.asarray(Wp, f32)
    wpc = np.ascontiguousarray(
        np.concatenate([wp[:, :, 1].T, wp[:, :, 0].T], axis=0)).astype(f16)
    bpc = np.ascontiguousarray(np.asarray(bp, f32).reshape(NCHUNK, P))

    def conv_t(W):
        # W [co, ci, k] -> [cic, ci, k, coc, co] (partition-major, contiguous)
        a = np.asarray(W, f32).reshape(NCHUNK, P, NCHUNK, P, 3)
        return np.ascontiguousarray(a.transpose(2, 3, 4, 0, 1)).astype(f16)

    w1t, w2t, w3t = conv_t(W1), conv_t(W2), conv_t(W3)
    b1c = np.ascontiguousarray(np.asarray(b1, f32).reshape(NCHUNK, P))
    b2c = np.ascontiguousarray(np.asarray(b2, f32).reshape(NCHUNK, P))
    b3c = np.ascontiguousarray(np.asarray(b3, f32).reshape(NCHUNK, P))
    # Wf1 [512, 30976] -> [cc, c, l, f] fp16 (contiguous per-partition DMA)
    wf1t = np.ascontiguousarray(
        np.asarray(Wf1, f32).reshape(F1, NCHUNK, P, LF)
        .transpose(1, 2, 3, 0)).astype(f16)
    bf1r = np.ascontiguousarray(np.asarray(bf1, f32).reshape(1, F1)).astype(f16)
    onesr = np.ones((1, P), f16)
    wf2p = np.zeros((4, P, P), f16)
    wf2p[:, :, 0] = np.asarray(Wf2, f32).reshape(4, P)
    wf2b = np.ascontiguousarray(
        np.broadcast_to(np.asarray(Wf2, f32).reshape(1, F1), (P, F1))).astype(f16)
    bf2s = np.asarray(bf2, f32).reshape(1, 1)
    identr = np.eye(P, dtype=f16)

    shared = dict(wpc=wpc, bpc=bpc, w1t=w1t, w2t=w2t, w3t=w3t,
                  b1c=b1c, b2c=b2c, b3c=b3c, wf1t=wf1t, bf1r=bf1r,
                  wf2p=wf2p, wf2b=wf2b, bf2s=bf2s, onesr=onesr,
                  identr=identr)
    # x packed j-major at per-sample stride S0=127 (position i' = orig i'+1):
    # xfull[j, b, i'] = x[b, i'+1, j] for j<64 (shifting half),
    # = x[b, 0, j-64] for j>=64 (x0 replicated over i') -> contiguous DMA
    xr = np.asarray(x, f32).reshape(N_CORES, BCORE, CL, IL).astype(f16)
    top = xr[:, :, 1:, :].transpose(0, 3, 1, 2)                     # [nc,j,b,127]
    bot = np.broadcast_to(xr[:, :, 0, :].transpose(0, 2, 1)[:, :, :, None],
                          top.shape)
    xfull = np.ascontiguousarray(np.concatenate([top, bot], axis=1))
    return [dict(xfull=xfull[i], **shared) for i in range(N_CORES)]


def kernel(x, Wp, bp, W1, b1, W2, b2, W3, b3, Wf1, bf1, Wf2, bf2,
           trace=False, **run_kwargs):
    nc = _get_nc()
    in_maps = prep_inputs(x, Wp, bp, W1, b1, W2, b2, W3, b3, Wf1, bf1, Wf2, bf2)
    res = run_bass_kernel_spmd(nc, in_maps, core_ids=list(range(N_CORES)),
                               trace=trace, **run_kwargs)
    out = np.concatenate([res.results[i]["y"] for i in range(N_CORES)], axis=0)
    kernel.last_results = res
    return out.astype(np.float32)


kernel.last_results = None


# revision 20
# speedup vs baseline: 1.0038x; 1.0038x over previous
"""Trainium2 Bass kernel for nn_CNN3_P (dense_cnn), 8-core data parallel.

Network (per sample):
  x [128,64] -> pairwise conv -> relu -> [256,127]
  -> conv1d k3 (x3, relu) -> [256,121] -> FC 30976->512 relu -> FC 512->1

Strategy: batch 2048 split 256/core. Channels on partitions (2 chunks of
128); all layers run on a flat [128, T*stride] layout where the K=3 conv
shifts are plain column offsets; boundary columns hold garbage that
never reaches valid outputs. All matmuls in fp16 (1 cyc/row on the PE),
PSUM accumulates fp32. Conv3 output is stored (s, l)-major so FC1's
stationary operands are contiguous.

Changes vs the 652.7us flat-128 version (now ~640us):

Progressive stride compaction: each
evac writes only the valid positions at a narrower per-sample stride
(x/pairwise 127 -> conv1 125 -> conv2 123 -> conv3 121), so conv
matmuls never re-process boundary-garbage columns. Saves ~27K PE
column-cycles (~11us) over the uniform stride-128 layout. Conv
evacuations alternate DVE/ACT by output chunk so both nt=1 chunks
evacuate concurrently at pipeline fill. The FC2 endgame ships the
sc=0 y-half early as a single-descriptor row DMA (a partition-major
[128,1] DMA is 128 descriptors whose completion bookkeeping adds
~3us to the runtime finalize window), and sc=0 defers fewer trailing
FC1 l-values (HOLD0=6 vs HOLD1=10) so its DVE dot-product chain
hides entirely under sc=1's deferred matmuls.

Schedule notes: x is packed host-side into one j-major dram tensor so
each t-tile is a single fully contiguous DMA (the first tile's DMA is
split across two queues to shorten the critical path to the first
matmul); all weight dram layouts are partition-major so weight DMAs
are contiguous; the Wf1 stream pool (bufs=4) is opened alongside the
conv pools in disjoint SBUF, so its first groups prefetch during the
conv phase instead of stalling the PE ~6us at the conv->FC transition
and the stream stays ahead of FC1; pairwise(1) is hoisted before the
conv loop to cover the conv1-weight DMA; the sc=0 FC1 psum group
closes ~2us before sc=1 (HOLD trailing l-values) so FC2-sc0 overlaps
FC1-sc1 matmuls, and the h4 activation is sliced per-128 so the FC2
transposes pipeline off the PE sooner. The FC2 identity is DMA'd on
the gpsimd queue instead of built with gpsimd DIRECT2D ops, which
otherwise run at the head of the gpsimd queue and delay the first
x-tile DMA.

fp8 was evaluated and rejected: DoubleRow fp8 matmuls cost the same
wall time as fp16 for the same output columns (2x MACs, measured
218ns for 256x128x512), and e4m3's ~5% rms/operand error gives ~12%
output error vs the 2e-2 gate, so neither plain nor hi/lo compensated
fp8 can win. Winograd F(2,3) was evaluated and rejected: the output
transform's tensor_tensor adds can only run on the DVE (ACT is
single-input), and at 1 elem/cyc/lane x 0.96 GHz on PSUM operands the
added DVE work exceeds the PE cycles saved.
"""
import os
import sys

for _p in ('/opt/trn_rl_repo', '/root/.axon_site/_ro/trn_rl_repo'):
    if os.path.isdir(_p) and _p not in sys.path:
        sys.path.insert(0, _p)

import numpy as np

import concourse.bacc as bacc
import concourse.mybir as mybir
import concourse.tile as tile
from concourse.bass_utils import run_bass_kernel_spmd

F32 = mybir.dt.float32
F16 = mybir.dt.float16

P = 128
CL = 128          # context length
IL = 64           # inst length
PC = 256          # channels (all layers)
NCHUNK = 2        # channel chunks of 128
LF = 121          # conv3 valid positions
F1 = 512
N_CORES = 8
B = 2048
BCORE = B // N_CORES      # 256
T = 8                     # samples per conv sub-tile
NT = BCORE // T           # 32
# per-sample strides after each stage's compacting evacuation
S0 = 127                  # x pack / pairwise output
S1 = 125                  # conv1 output
S2 = 123                  # conv2 output
S3 = 121                  # conv3 output (== LF)
H0 = T * S0               # 1016
H1 = T * S1               # 1000
H2 = T * S2               # 984
W0 = 4 * S0               # 508  pairwise/conv1 psum width
W1 = 4 * S1               # 500  conv2 psum width
W2 = 4 * S2               # 492  conv3 psum width
NTC = 2                   # psum tiles per t-tile
SPT = 4                   # samples per psum tile
SC = BCORE // P           # 2 sample chunks of 128 for FC
GL = 11                   # l-slices per Wf1 DMA group (121 = 11*11)


def build_nc():
    nc = bacc.Bacc("TRN2", target_bir_lowering=False, debug=False)

    x_d = nc.dram_tensor("xfull", [P, BCORE, S0], F16, kind="ExternalInput")
    wpc_d = nc.dram_tensor("wpc", [P, PC], F16, kind="ExternalInput")
    bp_d = nc.dram_tensor("bpc", [NCHUNK, P], F32, kind="ExternalInput")
    wc_d = [nc.dram_tensor(f"w{i}t", [NCHUNK, P, 3, NCHUNK, P], F16,
                           kind="ExternalInput") for i in (1, 2, 3)]
    bc_d = [nc.dram_tensor(f"b{i}c", [NCHUNK, P], F32, kind="ExternalInput")
            for i in (1, 2, 3)]
    wf1_d = nc.dram_tensor("wf1t", [NCHUNK, P, LF, F1], F16, kind="ExternalInput")
    bf1_d = nc.dram_tensor("bf1r", [1, F1], F16, kind="ExternalInput")
    ones_d = nc.dram_tensor("onesr", [1, P], F16, kind="ExternalInput")
    wf2_d = nc.dram_tensor("wf2p", [4, P, P], F16, kind="ExternalInput")
    wf2b_d = nc.dram_tensor("wf2b", [P, F1], F16, kind="ExternalInput")
    bf2_d = nc.dram_tensor("bf2s", [1, 1], F32, kind="ExternalInput")
    ident_d = nc.dram_tensor("identr", [P, P], F16, kind="ExternalInput")
    y_d = nc.dram_tensor("y", [BCORE, 1], F32, kind="ExternalOutput")

    RELU = mybir.ActivationFunctionType.Relu

    with tile.TileContext(nc) as tc:
        with tc.tile_pool(name="const", bufs=1) as cpool, \
             tc.tile_pool(name="h3c", bufs=1) as h3pool, \
             tc.tile_pool(name="wf1", bufs=4) as wfpool:
            # --- constants / weights, resident all kernel ---
            wpc = cpool.tile([P, PC], F16)
            nc.sync.dma_start(wpc[:], wpc_d.ap())
            bp = cpool.tile([P, NCHUNK], F32)
            nc.sync.dma_start(bp[:], bp_d.ap().rearrange("c p -> p c"))
            bf1 = cpool.tile([1, F1], F16)
            nc.sync.dma_start(bf1[:], bf1_d.ap())
            ones = cpool.tile([1, P], F16)
            nc.sync.dma_start(ones[:], ones_d.ap())
            # conv weights: per layer, per ci-chunk: [ci, (k, coc, co)]
            wconv = []
            for i in range(3):
                tiles = []
                for cic in range(NCHUNK):
                    w = cpool.tile([P, 3 * NCHUNK * P], F16, tag=f"w{i}_{cic}")
                    nc.sync.dma_start(
                        w[:].rearrange("p (k b c) -> p k b c", k=3, b=NCHUNK),
                        wc_d[i].ap()[cic])
                    tiles.append(w)
                wconv.append(tiles)
            bconv = []
            for i in range(3):
                bt = cpool.tile([P, NCHUNK], F32, tag=f"bc{i}")
                nc.sync.dma_start(bt[:], bc_d[i].ap().rearrange("c p -> p c"))
                bconv.append(bt)
            bf2 = cpool.tile([1, 1], F32)
            nc.sync.dma_start(bf2[:], bf2_d.ap())

            # persistent conv3 output, fp16, (s, l)-major: col = s*S3 + l
            h3c = [h3pool.tile([P, BCORE * S3], F16, tag=f"h3c{cc}", name=f"h3c{cc}")
                   for cc in range(NCHUNK)]
            h3v = [h.rearrange("p (s l) -> p s l", l=S3) for h in h3c]

            # ---------------- conv phase ----------------
            with tc.tile_pool(name="xt", bufs=3) as xtpool, \
                 tc.tile_pool(name="h", bufs=2) as hpool, \
                 tc.tile_pool(name="psP", bufs=2, space="PSUM") as psP, \
                 tc.tile_pool(name="ps1", bufs=2, space="PSUM") as ps1p, \
                 tc.tile_pool(name="ps2", bufs=2, space="PSUM") as ps2p, \
                 tc.tile_pool(name="ps3", bufs=2, space="PSUM") as ps3p:
                NTS = list(range(NTC - 1, -1, -1))   # nt=1 first: its consumers
                # don't cross the nt boundary, so they unblock earliest

                # HAM pre-warm: the PE clock idles throttled at 1.2 GHz and
                # only reaches 2.4 GHz after ~3.4us of sustained activity.
                # The first x/weight DMAs cannot land before ~9us (engine
                # boot + trigger latency), so burn that window on dummy
                # matmuls over a memset scratch tile (40 x N=128: starts
                # earliest and rides out the run-to-run jitter in when the
                # first x operands land; 8 x N=508 and 60 x N=128 both
                # measured worse -- late-landing runs idle past the HAM
                # MID window and re-throttle),
                # so the real matmuls start at full clock instead of paying
                # ~2.5us of half-rate warmup.
                with tc.high_priority():
                    scratch = cpool.tile([P, P], F16, tag="warm")
                    nc.gpsimd.memset(scratch[:], 0.0)
                    wps = psP.tile([P, W0], F32, tag="ps", name="warmps")
                    for _ in range(40):
                        nc.tensor.matmul(wps[:, 0:P], scratch[:], scratch[:],
                                         start=True, stop=True)

                def pairwise(t):
                    xt = xtpool.tile([P, H0], F16, tag="xt", name="xt")
                    if t == 0:
                        # first tile: split across the gpsimd + scalar
                        # queues (sync is busy with the weight triggers).
                        # The nt=1 half (samples 4-7) feeds the first
                        # matmul, so it rides gpsimd, which lands before
                        # the scalar queue clears its ACT_TABLE_LOAD.
                        xv = xt[:].rearrange("p (s i) -> p s i", i=S0)
                        nc.gpsimd.dma_start(xv[:, T // 2:T],
                                            x_d.ap()[:, T // 2:T, :])
                        nc.scalar.dma_start(xv[:, 0:T // 2],
                                            x_d.ap()[:, 0:T // 2, :])
                    else:
                        nc.gpsimd.dma_start(
                            xt[:].rearrange("p (s i) -> p s i", i=S0),
                            x_d.ap()[:, t * T:(t + 1) * T, :])
                    h0 = [hpool.tile([P, H0], F16, tag=f"h0_{cc}", bufs=4,
                                     name=f"h0_{cc}") for cc in range(NCHUNK)]
                    for nt in NTS:
                        for cc in range(NCHUNK):
                            ps = psP.tile([P, W0], F32, tag="ps", name="pwps")
                            sl_ = slice(nt * W0, (nt + 1) * W0)
                            nc.tensor.matmul(ps[:], wpc[:, cc * P:(cc + 1) * P],
                                             xt[:, sl_], start=True, stop=True)
                            nc.scalar.activation(h0[cc][:, sl_], ps[:],
                                                 RELU, bias=bp[:, cc:cc + 1])
                    return h0

                def conv_layer(hin, w_tiles, pool, hlen, wout, evac):
                    # group-outer: each psum group completes early so its
                    # evacuation overlaps the remaining groups' matmuls
                    for nt in NTS:
                        for co in range(NCHUNK):
                            ps = pool.tile([P, wout], F32,
                                           tag="ps", name=f"cps{co}_{nt}")
                            step = 0
                            for k in range(3):
                                for ci in range(NCHUNK):
                                    lhsT = w_tiles[ci][:, (k * NCHUNK + co) * P:
                                                       (k * NCHUNK + co + 1) * P]
                                    nk = min(wout, hlen - nt * wout - k)
                                    nc.tensor.matmul(
                                        ps[:, 0:nk], lhsT,
                                        hin[ci][:, nt * wout + k:
                                                nt * wout + k + nk],
                                        start=(step == 0), stop=(step == 5))
                                    step += 1
                            evac(co, nt, ps)

                h0_next = pairwise(0)
                # hoist pairwise(1): its matmuls give the PE ~0.9us of work
                # while the conv1 weight DMA finishes at kernel start
                h0_after = pairwise(1)
                for t in range(NT):
                    h0 = h0_next
                    h0_next = h0_after
                    # h1/h2 are produced and consumed within one t-iteration,
                    # so a single buffer suffices (h0 needs 3: pairwise runs
                    # two tiles ahead)
                    h1 = [hpool.tile([P, H1], F16, tag=f"h1_{cc}", bufs=1,
                                     name=f"h1_{cc}") for cc in range(NCHUNK)]

                    def evac1(co, nt, ps):
                        out = (h1[co][:].rearrange("p (s l) -> p s l", l=S1)
                               [:, nt * SPT:(nt + 1) * SPT, :])
                        src = (ps[:].rearrange("p (s l) -> p s l", l=S0)
                               [:, :, 0:S1])
                        if co == 0:
                            nc.vector.tensor_scalar(
                                out, src, bconv[0][:, co:co + 1], 0.0,
                                mybir.AluOpType.add, mybir.AluOpType.max)
                        else:
                            nc.scalar.activation(out, src, RELU,
                                                 bias=bconv[0][:, co:co + 1])
                    conv_layer(h0, wconv[0], ps1p, H0, W0, evac1)

                    # emit the next-but-one tile's pairwise here so its
                    # evacuations age a full tile before conv1 consumes them
                    if t + 2 < NT:
                        h0_after = pairwise(t + 2)

                    h2 = [hpool.tile([P, H2], F16, tag=f"h2_{cc}", bufs=1,
                                     name=f"h2_{cc}") for cc in range(NCHUNK)]

                    def evac2(co, nt, ps):
                        out = (h2[co][:].rearrange("p (s l) -> p s l", l=S2)
                               [:, nt * SPT:(nt + 1) * SPT, :])
                        src = (ps[:].rearrange("p (s l) -> p s l", l=S1)
                               [:, :, 0:S2])
                        if co == 0:
                            nc.vector.tensor_scalar(
                                out, src, bconv[1][:, co:co + 1], 0.0,
                                mybir.AluOpType.add, mybir.AluOpType.max)
                        else:
                            nc.scalar.activation(out, src, RELU,
                                                 bias=bconv[1][:, co:co + 1])
                    conv_layer(h1, wconv[1], ps2p, H1, W1, evac2)

                    def evac3(co, nt, ps, t=t):
                        s0_ = t * T + nt * SPT
                        out = h3v[co][:, s0_:s0_ + SPT, :]
                        src = (ps[:].rearrange("p (s l) -> p s l", l=S2)
                               [:, :, 0:S3])
                        if co == 0:
                            nc.scalar.activation(out, src, RELU,
                                                 bias=bconv[2][:, co:co + 1])
                        else:
                            nc.vector.tensor_scalar(
                                out, src, bconv[2][:, co:co + 1], 0.0,
                                mybir.AluOpType.add, mybir.AluOpType.max)
                    conv_layer(h2, wconv[2], ps3p, H2, W2, evac3)

            # ---------------- FC phase ----------------
            # (conv pools closed; wfpool stays open from before the conv
            # phase so its first group DMAs prefetch during conv)
            with tc.tile_pool(name="h4", bufs=1) as h4pool, \
                 tc.tile_pool(name="f2ps", bufs=2, space="PSUM") as f2pspool:
                # FC2-only constants (not needed until ~100us into the FC
                # phase), loaded into SBUF freed by the conv pools; gpsimd
                # queue (drained by now) so they don't sit behind the
                # blocked wf1 group DMAs on the sync queue
                wf2 = h4pool.tile([P, 4 * P], F16, tag="wf2")
                nc.gpsimd.dma_start(wf2[:].rearrange("p (f m) -> p f m", f=4),
                                    wf2_d.ap().rearrange("f p m -> p f m"))
                wf2b = h4pool.tile([P, F1], F16, tag="wf2b")
                nc.gpsimd.dma_start(wf2b[:], wf2b_d.ap())
                ident = h4pool.tile([P, P], F16, tag="ident")
                nc.gpsimd.dma_start(ident[:], ident_d.ap())
                ps_fc1 = [f2pspool.tile([P, F1], F32, tag=f"fc1ps{sc}", bufs=1,
                                        name=f"fc1ps{sc}") for sc in range(SC)]
                for sc in range(SC):
                    nc.tensor.matmul(ps_fc1[sc][:], ones[:], bf1[:],
                                     start=True, stop=False)
                # HOLD = trailing l-values run sc-serialized so sc=0's psum
                # closes early and its DVE FC2 chain mostly overlaps sc=1's
                # deferred matmuls. HOLD=14 (a window that fully covers the
                # ~2.4us chain) measured ~1us SLOWER than 10 — don't "fix"
                # the remaining transpose wait by widening this. sc=0
                # defers fewer (HOLD0) so its ~1.9us DVE chain starts
                # earlier and the ytp transpose is ready before sc=1's
                # deferred matmuls retire.
                HOLD = {0: 6, 1: 10}
                deferred = {0: [], 1: []}
                for cc in range(NCHUNK):
                    for lg in range(LF // GL):
                        rw = wfpool.tile([P, GL * F1], F16, tag="wf1")
                        nc.sync.dma_start(
                            rw[:].rearrange("p (l f) -> p l f", l=GL),
                            wf1_d.ap()[cc][:, lg * GL:(lg + 1) * GL, :])
                        for ll in range(GL):
                            l = lg * GL + ll
                            for sc in range(SC):
                                if (cc == NCHUNK - 1
                                        and l >= LF - HOLD[sc]):
                                    deferred[sc].append((cc, l, rw, ll))
                                    continue
                                nc.tensor.matmul(
                                    ps_fc1[sc][:],
                                    h3v[cc][:, sc * P:(sc + 1) * P, l],
                                    rw[:, ll * F1:(ll + 1) * F1],
                                    start=False, stop=False)
                ystage = h4pool.tile([1, BCORE], F32, tag="ystage")

                def fc1_tail(sc):
                    for i, (cc, l, rw, ll) in enumerate(deferred[sc]):
                        nc.tensor.matmul(
                            ps_fc1[sc][:],
                            h3v[cc][:, sc * P:(sc + 1) * P, l],
                            rw[:, ll * F1:(ll + 1) * F1],
                            start=False, stop=(i == len(deferred[sc]) - 1))

                # --- sc=0: FC2 dot product on the DVE; everything except
                # the tiny gather transpose overlaps sc=1's deferred
                # matmuls, and its y-half ships early as a single-
                # descriptor DMA (a partition-major [128,1] DMA is 128
                # descriptors whose completion bookkeeping adds ~3us to
                # the runtime finalize window -- don't) ---
                fc1_tail(0)
                h40 = h4pool.tile([P, F1], F16, tag="h4_0", name="h4_0")
                nc.scalar.activation(h40[:], ps_fc1[0][:], RELU)
                prod = h4pool.tile([P, F1], F32, tag="pr0")
                nc.vector.tensor_tensor(prod[:], h40[:], wf2b[:],
                                        mybir.AluOpType.mult)
                ysum = h4pool.tile([P, 1], F32, tag="ys0")
                nc.vector.tensor_reduce(ysum[:], prod[:], mybir.AxisListType.X,
                                        mybir.AluOpType.add)
                y16 = h4pool.tile([P, 1], F16, tag="y16")
                nc.vector.tensor_copy(y16[:], ysum[:])

                fc1_tail(1)
                # sc=0 gather: [128,1] -> [1,128] on the PE (~30ns, operands
                # ready long before sc=1's last matmul retires)
                ytp = f2pspool.tile([1, P], F16, tag="ytp", bufs=1)
                nc.tensor.transpose(ytp[:], y16[:], ident[:])
                nc.vector.tensor_scalar_add(ystage[:, 0:P], ytp[0:1, :], bf2[:])
                nc.sync.dma_start(y_d.ap().rearrange("b one -> one b")
                                  [:, 0:P], ystage[:, 0:P])
                # --- sc=1: FC2 via PE transposes (shortest tail chain); the
                # activation is sliced per-128, alternating ACT/DVE so two
                # slices evacuate concurrently, and each transpose starts
                # as soon as its slice is ready ---
                h41 = h4pool.tile([P, F1], F16, tag="h4_1", name="h4_1")
                h4t = h4pool.tile([P, 4 * P], F16, tag="h4t_1", name="h4t_1")
                for fc in range(4):
                    sl_ = slice(fc * P, (fc + 1) * P)
                    nc.scalar.activation(h41[:, sl_], ps_fc1[1][:, sl_], RELU)
                    tp = f2pspool.tile([P, P], F16, tag="fc2tp", bufs=2)
                    nc.tensor.transpose(tp[:], h41[:, sl_], ident[:])
                    nc.vector.tensor_copy(h4t[:, sl_], tp[:])
                po = f2pspool.tile([P, P], F32, tag="fc2ps", bufs=1)
                for fc in range(4):
                    nc.tensor.matmul(po[:], wf2[:, fc * P:(fc + 1) * P],
                                     h4t[:, fc * P:(fc + 1) * P],
                                     start=(fc == 0), stop=(fc == 3))
                nc.vector.tensor_scalar_add(ystage[:, P:2 * P],
                                            po[0:1, :], bf2[:])
                nc.sync.dma_start(y_d.ap().rearrange("b one -> one b")
                                  [:, P:2 * P], ystage[:, P:2 * P])

    nc.compile()
    return nc


_NC_CACHE = None


def _get_nc():
    global _NC_CACHE
    if _NC_CACHE is None:
        _NC_CACHE = build_nc()
    return _NC_CACHE


def prep_inputs(x, Wp, bp, W1, b1, W2, b2, W3, b3, Wf1, bf1, Wf2, bf2):
    """Host-side shard + weight re-layout. Returns per-core input maps."""
    f32, f16 = np.float32, np.float16
    wp = np.asarray(Wp, f32)
    wpc = np.ascontiguousarray(
        np.concatenate([wp[:, :, 1].T, wp[:, :, 0].T], axis=0)).astype(f16)
    bpc = np.ascontiguousarray(np.asarray(bp, f32).reshape(NCHUNK, P))

    def conv_t(W):
        # W [co, ci, k] -> [cic, ci, k, coc, co] (partition-major, contiguous)
        a = np.asarray(W, f32).reshape(NCHUNK, P, NCHUNK, P, 3)
        return np.ascontiguousarray(a.transpose(2, 3, 4, 0, 1)).astype(f16)

    w1t, w2t, w3t = conv_t(W1), conv_t(W2), conv_t(W3)
    b1c = np.ascontiguousarray(np.asarray(b1, f32).reshape(NCHUNK, P))
    b2c = np.ascontiguousarray(np.asarray(b2, f32).reshape(NCHUNK, P))
    b3c = np.ascontiguousarray(np.asarray(b3, f32).reshape(NCHUNK, P))
    # Wf1 [512, 30976] -> [cc, c, l, f] fp16 (contiguous per-partition DMA)
    wf1t = np.ascontiguousarray(
        np.asarray(Wf1, f32).reshape(F1, NCHUNK, P, LF)
        .transpose(1, 2, 3, 0)).astype(f16)
    bf1r = np.ascontiguousarray(np.asarray(bf1, f32).reshape(1, F1)).astype(f16)
    onesr = np.ones((1, P), f16)
    wf2p = np.zeros((4, P, P), f16)
    wf2p[:, :, 0] = np.asarray(Wf2, f32).reshape(4, P)
    wf2b = np.ascontiguousarray(
        np.broadcast_to(np.asarray(Wf2, f32).reshape(1, F1), (P, F1))).astype(f16)
    bf2s = np.asarray(bf2, f32).reshape(1, 1)
    identr = np.eye(P, dtype=f16)

    shared = dict(wpc=wpc, bpc=bpc, w1t=w1t, w2t=w2t, w3t=w3t,
                  b1c=b1c, b2c=b2c, b3c=b3c, wf1t=wf1t, bf1r=bf1r,
                  wf2p=wf2p, wf2b=wf2b, bf2s=bf2s, onesr=onesr,
                  identr=identr)
    # x packed j-major at per-sample stride S0=127 (position i' = orig i'+1):
    # xfull[j, b, i'] = x[b, i'+1, j] for j<64 (shifting half),
    # = x[b, 0, j-64] for j>=64 (x0 replicated over i') -> contiguous DMA
    xr = np.asarray(x, f32).reshape(N_CORES, BCORE, CL, IL).astype(f16)
    top = xr[:, :, 1:, :].transpose(0, 3, 1, 2)                     # [nc,j,b,127]
    bot = np.broadcast_to(xr[:, :, 0, :].transpose(0, 2, 1)[:, :, :, None],
                          top.shape)
    xfull = np.ascontiguousarray(np.concatenate([top, bot], axis=1))
    return [dict(xfull=xfull[i], **shared) for i in range(N_CORES)]


def kernel(x, Wp, bp, W1, b1, W2, b2, W3, b3, Wf1, bf1, Wf2, bf2,
           trace=False, **run_kwargs):
    nc = _get_nc()
    in_maps = prep_inputs(x, Wp, bp, W1, b1, W2, b2, W3, b3, Wf1, bf1, Wf2, bf2)
    res = run_bass_kernel_spmd(nc, in_maps, core_ids=list(range(N_CORES)),
                               trace=trace, **run_kwargs)
    out = np.concatenate([res.results[i]["y"] for i in range(N_CORES)], axis=0)
    kernel.last_results = res
    return out.astype(np.float32)


kernel.last_results = None
